# revision 19
# baseline (speedup 1.0000x reference)
"""GCN message-passing kernel for 8 Trainium2 NeuronCores (Bass/Tile).

Computes (matching the jax reference):
    h = x @ W_conv                      [N, H]
    node_embed = leaky_relu(D^-1/2 (A+I) D^-1/2 h + b_conv)
    out = sigmoid(leaky(cat(e[i], e[j]) @ W1 + b1) @ W2 + b2)

Distribution: nodes are sharded over the 8 cores (dst-sharded aggregation
with an AllGather of the scaled features g = dinv * h).

Aggregation strategy ("slab" design): nodes are relabeled so each int16
gather range (25088 pair-packed fp16 rows) ends with guaranteed-zero pad
rows. Each dst tile owns fixed per-node slot slabs (node = partition,
slot = free dim); a bulk dma_gather fills each group's slabs in edge
order, a predicated copy selects the pair-parity half (host-built masks,
pads masked to the zero row), and a log-depth tree of vector adds does
the segmented sum — no per-chunk one-hot matmuls on the hot path.
Per-node degree overflow beyond the slab cap goes through the original
one-hot TensorEngine scatter (cheap: few % of edges).
The pair-MLP head reuses gather + one-hot-permute machinery.
"""

import re

import numpy as np

import concourse.bass as bass
import concourse.bacc as bacc
import concourse.mybir as mybir
import concourse.tile as tile
from concourse import library_config
from concourse.bass_utils import run_bass_kernel_spmd

NC = 8
N_NODES = 100000
F_IN = 256
H = 64
NEG = 0.01

P = 128                    # partitions / tile height
TILES = 98                 # node tiles per core
SHARD = TILES * P          # 12544 nodes per core
NPAD = NC * SHARD          # 100352
GROUP = 4                  # node tiles per slab group
OVGROUP = 4                # tiles per overflow scatter group (PSUM: 4 tags x 2)
NBUCKET = 4                # (src range) x (src parity) for one-hot path
PGROUP = 4                 # pair slot-tiles per gather group

HR = NPAD // 4             # 25088: int16-addressable pair rows per range
RANGE_B = 2 * HR * 2 // 2  # node id where range B starts = 50176
SPLIT = 50000              # real nodes below SPLIT -> range A ids
ZERO_ROW = 25000           # pair row (local to each range) guaranteed zero


def _relabel(n):
    """Real node id -> padded id with per-range zero rows at each tail."""
    return n + (n >= SPLIT) * (RANGE_B - SPLIT)


def _wrap_idx_window(idx):
    """int array [W] (W % 16 == 0) -> [128, W//16] int16 wrapped/replicated."""
    w = idx.reshape(-1, 16).T.astype(np.int16)
    return np.tile(w, (8, 1))


def _scatter_sched(core, tl, loc, node, ntiles, group_sz, nbucket=NBUCKET):
    """Cross-core-uniform one-hot gather/scatter schedule (overflow + pairs).

    Each item is gathered from pair-packed row (node>>1) and scatter-added
    to column `loc` of tile `tl` on core `core`. nbucket=4 buckets by
    (src range, src parity); nbucket=2 by src range only (parity handled
    downstream via the returned parity mask).
    Returns (sched, idx_i16 [NC,128,totidx//16], loc_f16, par_f16).
    """
    ngroups = (ntiles + group_sz - 1) // group_sz
    if nbucket == 4:
        bucket = 2 * (node >= 2 * HR).astype(np.int64) + (node & 1)
        rng_of = bucket >= 2
    else:
        assert nbucket == 2
        bucket = (node >= 2 * HR).astype(np.int64)
        rng_of = bucket >= 1
    grp = tl // group_sz

    tid = ((core * ngroups + grp) * nbucket + bucket) * ntiles + tl
    n_bins = NC * ngroups * nbucket * ntiles
    cnt = np.bincount(tid, minlength=n_bins).reshape(NC, ngroups, nbucket, ntiles)
    K = (cnt + P - 1) // P
    K = K.max(axis=0)                       # [ngroups, nbucket, ntiles]
    per_tile = K.sum(axis=1)                # [ngroups, ntiles]
    for t in range(ntiles):
        g = t // group_sz
        if per_tile[g, t] == 0:
            K[g, 0, t] = 1

    chunk_meta = []
    win_meta = []
    for g in range(ngroups):
        t0, t1 = g * group_sz, min((g + 1) * group_sz, ntiles)
        for b in range(nbucket):
            c0 = len(chunk_meta)
            for t in range(t0, t1):
                for _ in range(K[g, b, t]):
                    chunk_meta.append((g, b, t))
            win_meta.append((g, b, c0, len(chunk_meta) - c0))
    totchunks = len(chunk_meta)
    totidx = totchunks * P

    slot_base = np.zeros((ngroups, nbucket, ntiles), np.int64)
    acc = 0
    for g in range(ngroups):
        t0, t1 = g * group_sz, min((g + 1) * group_sz, ntiles)
        for b in range(nbucket):
            for t in range(t0, t1):
                slot_base[g, b, t] = acc
                acc += K[g, b, t] * P
    assert acc == totidx

    loc_arr = np.full((NC, totidx), 255, np.int64)
    par_arr = np.zeros((NC, totidx), np.int64)
    pidx_arr = np.zeros((NC, totidx), np.int64)
    order = np.lexsort((tl, bucket, grp, core))
    so_core = core[order]
    so_tid = tid[order]
    so_node = node[order]
    so_loc = loc[order]
    so_bucket = bucket[order]
    so_grp = grp[order]
    so_t = tl[order]
    starts = np.r_[0, np.flatnonzero(np.diff(so_tid)) + 1]
    run_ids = np.zeros(len(so_tid), np.int64)
    run_ids[starts[1:]] = 1
    run_ids = np.cumsum(run_ids)
    rank = np.arange(len(so_tid)) - starts[run_ids]
    slot = slot_base[so_grp, so_bucket, so_t] + rank
    if nbucket == 4:
        pair_local = (so_node >> 1) - (so_bucket >= 2) * HR
    else:
        pair_local = (so_node >> 1) - (so_bucket >= 1) * HR
    loc_arr[so_core, slot] = so_loc
    par_arr[so_core, slot] = 1 - (so_node & 1)
    pidx_arr[so_core, slot] = pair_local

    loc_f16 = np.zeros((NC, P, totchunks), np.float16)
    par_f16 = np.zeros((NC, P, totchunks), np.int8)
    idx_i16 = np.zeros((NC, P, totidx // 16), np.int16)
    for c in range(NC):
        loc_f16[c] = loc_arr[c].reshape(totchunks, P).T.astype(np.float16)
        par_f16[c] = par_arr[c].reshape(totchunks, P).T.astype(np.float16)
        for (g, b, c0, nch) in win_meta:
            if nch == 0:
                continue
            lo, hi = c0 * P, (c0 + nch) * P
            idx_i16[c][:, lo // 16: hi // 16] = _wrap_idx_window(pidx_arr[c, lo:hi])

    sched = {
        "chunk_meta": chunk_meta,
        "win_meta": win_meta,
        "totchunks": totchunks,
        "totidx": totidx,
        "ntiles": ntiles,
        "ngroups": ngroups,
        "group_sz": group_sz,
        "nbucket": nbucket,
    }
    return sched, idx_i16, loc_f16, par_f16


def _slab_sched(src_id, dst_id, cap):
    """Per-(node,range) slot-slab schedule with per-node cap.

    Returns (slab dict, idxA, idxB, mskA, mskB, overflow edge arrays).
    idx*: [NC, P, tot*//16] int16; msk*: [NC, P, totchunks*] fp16.
    """
    core = dst_id // SHARD
    nl = dst_id % SHARD            # node local 0..SHARD-1
    tl = nl // P
    part = nl % P
    plane = (src_id >= RANGE_B).astype(np.int64)
    prow = (src_id >> 1) - plane * HR
    parity = src_id & 1

    # rank of each edge within its (core, node, plane) run
    key = (core * SHARD + nl) * 2 + plane
    order = np.argsort(key, kind="stable")
    so_key = key[order]
    starts = np.r_[0, np.flatnonzero(np.diff(so_key)) + 1]
    run_ids = np.zeros(len(so_key), np.int64)
    run_ids[starts[1:]] = 1
    run_ids = np.cumsum(run_ids)
    rank_sorted = np.arange(len(so_key)) - starts[run_ids]
    rank = np.empty(len(so_key), np.int64)
    rank[order] = rank_sorted

    in_slab = rank < cap
    # per-group slab heights K_A/K_B: max capped per-node count, >= 1
    ngroups = (TILES + GROUP - 1) // GROUP
    cnt = np.zeros((NC, SHARD, 2), np.int64)
    np.add.at(cnt, (core, nl, plane), 1)
    capped = np.minimum(cnt, cap)               # [NC, SHARD, 2]
    capped = capped.reshape(NC, TILES, P, 2)
    Kt = capped.max(axis=(0, 2))                # [TILES, 2]
    KA = np.zeros(ngroups, np.int64)
    KB = np.zeros(ngroups, np.int64)
    gsz = []
    for g in range(ngroups):
        t0, t1 = g * GROUP, min((g + 1) * GROUP, TILES)
        gsz.append(t1 - t0)
        KA[g] = max(1, Kt[t0:t1, 0].max())
        KB[g] = max(1, Kt[t0:t1, 1].max())

    baseA = np.concatenate([[0], np.cumsum([gsz[g] * KA[g] for g in range(ngroups)])])
    baseB = np.concatenate([[0], np.cumsum([gsz[g] * KB[g] for g in range(ngroups)])])
    totA, totB = int(baseA[-1]), int(baseB[-1])

    idx_arr = [np.full((NC, totA * P), ZERO_ROW, np.int64),
               np.full((NC, totB * P), ZERO_ROW, np.int64)]
    msk_arr = [np.zeros((NC, totA * P), np.int8),
               np.zeros((NC, totB * P), np.int8)]

    sl = in_slab
    K_of = np.where(plane[sl] == 0, KA[tl[sl] // GROUP], KB[tl[sl] // GROUP])
    base_of = np.where(plane[sl] == 0, baseA[tl[sl] // GROUP], baseB[tl[sl] // GROUP])
    chunk = base_of + (tl[sl] % GROUP) * K_of + rank[sl]
    pos = chunk * P + part[sl]
    pl = plane[sl]
    for pp in (0, 1):
        m = pl == pp
        idx_arr[pp][core[sl][m], pos[m]] = prow[sl][m]
        msk_arr[pp][core[sl][m], pos[m]] = (1 - parity[sl][m]).astype(np.int8)

    idxA = np.zeros((NC, P, totA * P // 16), np.int16)
    idxB = np.zeros((NC, P, totB * P // 16), np.int16)
    mskA = np.zeros((NC, P, totA), np.int8)
    mskB = np.zeros((NC, P, totB), np.int8)
    win_list = [[], []]
    for g in range(ngroups):
        win_list[0].append((int(baseA[g]), int(gsz[g] * KA[g])))
        win_list[1].append((int(baseB[g]), int(gsz[g] * KB[g])))
    for c in range(NC):
        mskA[c] = msk_arr[0][c].reshape(totA, P).T
        mskB[c] = msk_arr[1][c].reshape(totB, P).T
        for (arr, out, tot) in ((idx_arr[0], idxA, totA), (idx_arr[1], idxB, totB)):
            pass
        for pp, (arr, out) in enumerate(((idx_arr[0], idxA), (idx_arr[1], idxB))):
            for (c0, nch) in win_list[pp]:
                if nch == 0:
                    continue
                lo, hi = c0 * P, (c0 + nch) * P
                out[c][:, lo // 16: hi // 16] = _wrap_idx_window(arr[c, lo:hi])

    slab = {
        "ngroups": ngroups,
        "gsz": gsz,
        "KA": KA.tolist(),
        "KB": KB.tolist(),
        "baseA": baseA.tolist(),
        "baseB": baseB.tolist(),
        "totA": totA,
        "totB": totB,
    }
    ov = ~in_slab
    return slab, idxA, idxB, mskA, mskB, (core[ov], tl[ov], part[ov], src_id[ov])


def _prep(inputs):
    x = np.asarray(inputs["x"], np.float32)
    edge_index = np.asarray(inputs["edge_index"], np.int64)
    index = np.asarray(inputs["index"], np.int64)
    W_conv = np.asarray(inputs["W_conv"], np.float32)
    b_conv = np.asarray(inputs["b_conv"], np.float32)
    W1 = np.asarray(inputs["W1"], np.float32)
    b1 = np.asarray(inputs["b1"], np.float32)
    W2 = np.asarray(inputs["W2"], np.float32)
    b2 = np.asarray(inputs["b2"], np.float32)

    n = x.shape[0]
    ids = _relabel(np.arange(n, dtype=np.int64))
    src = _relabel(edge_index[0].astype(np.int64))
    dst = _relabel(edge_index[1].astype(np.int64))
    src = np.concatenate([src, ids])
    dst = np.concatenate([dst, ids])

    deg = np.bincount(dst, minlength=NPAD).astype(np.float32)
    deg[deg == 0] = 1.0

    slab, eidxA, eidxB, emskA, emskB, ovedges = _slab_sched(src, dst, cap=7)
    ov_core, ov_tl, ov_part, ov_src = ovedges
    esched, oidx, oloc, opar = _scatter_sched(
        core=ov_core, tl=ov_tl, loc=ov_part, node=ov_src,
        ntiles=TILES, group_sz=OVGROUP, nbucket=2)

    # pair stream: per core PB pairs; side slots [xi: 0..PB) [xj: PB..2PB)
    B = index.shape[0]
    PB = B // NC
    assert PB % P == 0
    PCH = PB // P
    pair_global = np.arange(B, dtype=np.int64)
    pcore = pair_global // PB
    plocal = pair_global % PB
    s_core = np.concatenate([pcore, pcore])
    s_slot = np.concatenate([plocal, PB + plocal])
    s_node = _relabel(np.concatenate([index[:, 0], index[:, 1]]).astype(np.int64))
    psched, pidx, ploc, _ = _scatter_sched(
        core=s_core, tl=s_slot // P, loc=s_slot % P, node=s_node,
        ntiles=2 * PCH, group_sz=PGROUP)

    xpad = np.zeros((NPAD, F_IN), np.float32)
    xpad[ids] = x
    xT = xpad.T.astype(np.float16)
    xT_shards = [
        np.ascontiguousarray(
            xT[:, c * SHARD:(c + 1) * SHARD].reshape(2, P, SHARD).transpose(1, 0, 2)
        ) for c in range(NC)
    ]
    deg_sb = [
        np.ascontiguousarray(
            deg[c * SHARD:(c + 1) * SHARD].reshape(TILES, P).T
        ) for c in range(NC)
    ]
    bcd = (np.sqrt(deg)[:, None] * b_conv[None, :]).astype(np.float16)
    bcd_sh = [
        np.ascontiguousarray(
            bcd[c * SHARD:(c + 1) * SHARD].reshape(TILES, P, H)
            .transpose(1, 0, 2)
        ) for c in range(NC)
    ]

    consts = {
        "wc": np.ascontiguousarray(
            W_conv.reshape(2, P, H).transpose(1, 0, 2)).astype(np.float16),
        "bconvb": np.broadcast_to(b_conv, (P, H)).astype(np.float32).copy(),
        "iota": np.broadcast_to(np.arange(P, dtype=np.float16), (P, P)).copy(),
        "ident": np.eye(P, dtype=np.float16),
        "w1": W1.astype(np.float16),
        "b1": b1.reshape(16, 1).astype(np.float32),
        "w2": W2.astype(np.float32),
        "b2t": b2.reshape(1, 1).astype(np.float32),
    }
    sched = {"edge": esched, "pair": psched, "slab": slab, "PCH": PCH}
    in_maps = []
    for c in range(NC):
        m = {
            "xt": xT_shards[c],
            "deg": deg_sb[c],
            "bcd": bcd_sh[c],
            "eidxa": eidxA[c],
            "eidxb": eidxB[c],
            "emska": emskA[c],
            "emskb": emskB[c],
            "odstloc": oloc[c],
            "oparity": opar[c],
            "ogidx": oidx[c],
            "pdstloc": ploc[c],
            "pgidx": pidx[c],
        }
        m.update(consts)
        in_maps.append(m)
    return in_maps, sched


def _emit_scatter(nc, dt, src_pairs, idx_dram, loc_sb, iota, sched,
                  pools, consume, prefix):
    """One-hot gather/scatter path (overflow + pairs): gather pair rows per
    window, build one-hot, matmul-accumulate per dst tile in PSUM, then hand
    each finished tile to consume."""
    widxp, msgp, ohp, accp = pools
    chunk_meta = sched["chunk_meta"]
    win_meta = sched["win_meta"]
    ngroups = sched["ngroups"]
    ntiles = sched["ntiles"]
    group_sz = sched["group_sz"]

    first_chunk = {}
    last_chunk = {}
    for ci, (g, b, t) in enumerate(chunk_meta):
        first_chunk.setdefault(t, ci)
        last_chunk[t] = ci

    acc_tiles = {}
    for g in range(ngroups):
        t0, t1 = g * group_sz, min((g + 1) * group_sz, ntiles)
        for (gg, b, c0, nch) in win_meta:
            if gg != g or nch == 0:
                continue
            nidx = nch * P
            idxt = widxp.tile([P, nidx // 16], dt.int16, tag=f"{prefix}idx{b}",
                              name=f"{prefix}idx_g{g}b{b}")
            nc.sync.dma_start(
                idxt[:], idx_dram[:, c0 * P // 16:(c0 + nch) * P // 16])
            msg = msgp.tile([P, nch, P], dt.float16, tag=f"{prefix}msg{b}",
                            name=f"{prefix}msg_g{g}b{b}")
            src_ap = src_pairs if b < 2 else src_pairs[HR:, :]
            nc.gpsimd.dma_gather(
                msg[:], src_ap, idxt[:], nidx, nidx, P,
                single_packet=False, queue_num=0)
            oh = ohp.tile([P, nch, P], dt.float16, tag=f"{prefix}oh{b}",
                          name=f"{prefix}oh_g{g}b{b}")
            nc.vector.tensor_tensor(
                oh[:],
                loc_sb[:, c0:c0 + nch].unsqueeze(2).to_broadcast([P, nch, P]),
                iota[:, :].unsqueeze(1).to_broadcast([P, nch, P]),
                mybir.AluOpType.is_equal,
            )
            for ci in range(c0, c0 + nch):
                _, bb, t = chunk_meta[ci]
                if t not in acc_tiles:
                    acc_tiles[t] = accp.tile(
                        [P, H], dt.float32, tag=f"{prefix}acc{t % group_sz}",
                        name=f"{prefix}acc_t{t}")
                par = bb & 1
                nc.tensor.matmul(
                    acc_tiles[t][:],
                    lhsT=oh[:, ci - c0, :],
                    rhs=msg[:, ci - c0, par * H:(par + 1) * H],
                    start=(ci == first_chunk[t]),
                    stop=(ci == last_chunk[t]),
                )
        for t in range(t0, t1):
            consume(t, acc_tiles.pop(t))


def _build(sched, debug=False):
    dt = mybir.dt
    esched = sched["edge"]
    psched = sched["pair"]
    slab = sched["slab"]
    PCH = sched["PCH"]

    nc = bacc.Bacc("TRN2", target_bir_lowering=False, debug=False,
                   enable_asserts=False, num_devices=NC, num_swdge_queues=4)

    xt_in = nc.dram_tensor("xt", [P, 2, SHARD], dt.float16, kind="ExternalInput")
    deg_in = nc.dram_tensor("deg", [P, TILES], dt.float32, kind="ExternalInput")
    bcd_in = nc.dram_tensor("bcd", [P, TILES, H], dt.float16, kind="ExternalInput")
    eidxa_in = nc.dram_tensor("eidxa", [P, slab["totA"] * P // 16], dt.int16,
                              kind="ExternalInput")
    eidxb_in = nc.dram_tensor("eidxb", [P, slab["totB"] * P // 16], dt.int16,
                              kind="ExternalInput")
    emska_in = nc.dram_tensor("emska", [P, slab["totA"]], dt.int8,
                              kind="ExternalInput")
    emskb_in = nc.dram_tensor("emskb", [P, slab["totB"]], dt.int8,
                              kind="ExternalInput")
    odstloc_in = nc.dram_tensor("odstloc", [P, esched["totchunks"]], dt.float16,
                                kind="ExternalInput")
    oparity_in = nc.dram_tensor("oparity", [P, esched["totchunks"]], dt.int8,
                                kind="ExternalInput")
    ogidx_in = nc.dram_tensor("ogidx", [P, esched["totidx"] // 16], dt.int16,
                              kind="ExternalInput")
    pdstloc_in = nc.dram_tensor("pdstloc", [P, psched["totchunks"]], dt.float16,
                                kind="ExternalInput")
    pgidx_in = nc.dram_tensor("pgidx", [P, psched["totidx"] // 16], dt.int16,
                              kind="ExternalInput")
    wc_in = nc.dram_tensor("wc", [P, 2, H], dt.float16, kind="ExternalInput")
    bconvb_in = nc.dram_tensor("bconvb", [P, H], dt.float32, kind="ExternalInput")
    iota_in = nc.dram_tensor("iota", [P, P], dt.float16, kind="ExternalInput")
    ident_in = nc.dram_tensor("ident", [P, P], dt.float16, kind="ExternalInput")
    w1_in = nc.dram_tensor("w1", [P, 16], dt.float16, kind="ExternalInput")
    b1_in = nc.dram_tensor("b1", [16, 1], dt.float32, kind="ExternalInput")
    w2_in = nc.dram_tensor("w2", [16, 1], dt.float32, kind="ExternalInput")
    b2_in = nc.dram_tensor("b2t", [1, 1], dt.float32, kind="ExternalInput")
    outp = nc.dram_tensor("out", [PCH * P, 1], dt.float32, kind="ExternalOutput")
    if debug:
        dbg_e = nc.dram_tensor("dbg_e", [NPAD, H], dt.float16, kind="ExternalOutput")

    g_shard = nc.dram_tensor("g_shard", [SHARD, H], dt.float16)
    g_full = nc.dram_tensor("g_full", [NPAD, H], dt.float16, addr_space="Shared")
    e_shard = nc.dram_tensor("e_shard", [SHARD, H], dt.float16)
    e_full = nc.dram_tensor("e_full", [NPAD, H], dt.float16)

    g_pairs = g_full[:, :].rearrange("(r two) f -> r (two f)", two=2)
    e_pairs = e_full[:, :].rearrange("(r two) f -> r (two f)", two=2)

    with tile.TileContext(nc) as tc:
        nc.gpsimd.load_library(library_config.mlp)

        with (
            tc.tile_pool(name="const", bufs=1) as cpool,
            tc.tile_pool(name="dinvp", bufs=1) as dpool,
        ):
            wc_sb = cpool.tile([P, 2, H], dt.float16)
            nc.sync.dma_start(wc_sb[:], wc_in[:, :, :])
            bconvb = cpool.tile([P, H], dt.float32)
            nc.sync.dma_start(bconvb[:], bconvb_in[:, :])
            iota = cpool.tile([P, P], dt.float16)
            nc.sync.dma_start(iota[:], iota_in[:, :])
            ident = cpool.tile([P, P], dt.float16)
            nc.sync.dma_start(ident[:], ident_in[:, :])
            w1_sb = cpool.tile([P, 16], dt.float16)
            nc.sync.dma_start(w1_sb[:], w1_in[:, :])
            b1_sb = cpool.tile([16, 1], dt.float32)
            nc.sync.dma_start(b1_sb[:], b1_in[:, :])
            w2_sb = cpool.tile([16, 1], dt.float32)
            nc.sync.dma_start(w2_sb[:], w2_in[:, :])
            b2_sb = cpool.tile([1, 1], dt.float32)
            nc.sync.dma_start(b2_sb[:], b2_in[:, :])
            odstloc_sb = cpool.tile([P, esched["totchunks"]], dt.float16)
            nc.sync.dma_start(odstloc_sb[:], odstloc_in[:, :])
            oparity_sb = cpool.tile([P, esched["totchunks"]], dt.int8)
            nc.sync.dma_start(oparity_sb[:], oparity_in[:, :])
            emska_sb = cpool.tile([P, slab["totA"]], dt.int8)
            nc.sync.dma_start(emska_sb[:], emska_in[:, :])
            emskb_sb = cpool.tile([P, slab["totB"]], dt.int8)
            nc.sync.dma_start(emskb_sb[:], emskb_in[:, :])
            bcd_sb = cpool.tile([P, TILES, H], dt.float16)
            nc.sync.dma_start(bcd_sb[:], bcd_in[:, :, :])
            eidxa_sb = cpool.tile([P, slab["totA"] * P // 16], dt.int16)
            nc.sync.dma_start(eidxa_sb[:], eidxa_in[:, :])
            eidxb_sb = cpool.tile([P, slab["totB"] * P // 16], dt.int16)
            nc.sync.dma_start(eidxb_sb[:], eidxb_in[:, :])
            ogidx_sb = cpool.tile([P, esched["totidx"] // 16], dt.int16)
            nc.sync.dma_start(ogidx_sb[:], ogidx_in[:, :])

            deg_sb = dpool.tile([P, TILES], dt.float32)
            nc.sync.dma_start(deg_sb[:], deg_in[:, :])
            sq = dpool.tile([P, TILES], dt.float32)
            nc.scalar.activation(sq[:], deg_sb[:], mybir.ActivationFunctionType.Sqrt)
            dinv = dpool.tile([P, TILES], dt.float32)
            nc.vector.reciprocal(dinv[:], sq[:])

            # ---------------- phase A: g = (x @ W) * dinv ----------------
            XBLK = 16
            with (
                tc.tile_pool(name="xtp", bufs=2) as xtp,
                tc.tile_pool(name="hps", bufs=4, space="PSUM") as hps,
                tc.tile_pool(name="gsb", bufs=1) as gsbp,
            ):
                g_sb = gsbp.tile([P, TILES, H], dt.float16)
                for blk in range((TILES + XBLK - 1) // XBLK):
                    t0, t1 = blk * XBLK, min((blk + 1) * XBLK, TILES)
                    xt_sb = xtp.tile([P, 2, (t1 - t0) * P], dt.float16, tag="xt")
                    nc.sync.dma_start(xt_sb[:], xt_in[:, :, t0 * P: t1 * P])
                    for t in range(t0, t1):
                        h_ps = hps.tile([P, H], dt.float32)
                        for k in range(2):
                            nc.tensor.matmul(
                                h_ps[:],
                                lhsT=xt_sb[:, k, (t - t0) * P:(t - t0 + 1) * P],
                                rhs=wc_sb[:, k, :],
                                start=(k == 0), stop=(k == 1),
                            )
                        nc.scalar.activation(
                            g_sb[:, t, :], h_ps[:],
                            mybir.ActivationFunctionType.Copy,
                            scale=dinv[:, t:t + 1],
                        )
                nc.sync.dma_start(
                    g_shard[:, :].rearrange("(t p) f -> p t f", p=P),
                    g_sb[:, :, :],
                )

            nc.gpsimd.collective_compute(
                "AllGather", mybir.AluOpType.bypass,
                replica_groups=[list(range(NC))],
                ins=[g_shard[:, :].opt()],
                outs=[g_full[:, :].opt()],
            )

            # ---------------- phase C: slab aggregation per dst tile --------
            KA, KB = slab["KA"], slab["KB"]
            baseA, baseB = slab["baseA"], slab["baseB"]
            gsz = slab["gsz"]
            ngroups = slab["ngroups"]
            ov_sched = esched
            ov_first = {}
            ov_last = {}
            for ci, (g, b, t) in enumerate(ov_sched["chunk_meta"]):
                ov_first.setdefault(t, ci)
                ov_last[t] = ci

            with (
                tc.tile_pool(name="smsg", bufs=3) as smsgp,
                tc.tile_pool(name="ssel", bufs=3) as sselp,
                tc.tile_pool(name="semb", bufs=4) as sembp,
                tc.tile_pool(name="ovmsg", bufs=2) as ovmsgp,
                tc.tile_pool(name="ovsel", bufs=2) as ovselp,
                tc.tile_pool(name="ovoh", bufs=2) as ovohp,
                tc.tile_pool(name="ovacc", bufs=2, space="PSUM") as ovaccp,
            ):
                ov_acc = {}
                ov_ready = {}

                def emit_ov_gathers(g):
                    ov_ready[g] = []
                    for (gg, b, c0, nch) in ov_sched["win_meta"]:
                        if gg != g or nch == 0:
                            continue
                        nidx = nch * P
                        msg = ovmsgp.tile([P, nch, P], dt.float16, tag=f"omsg{b}",
                                          name=f"omsg_g{g}b{b}")
                        src_ap = g_pairs if b == 0 else g_pairs[HR:, :]
                        nc.gpsimd.dma_gather(
                            msg[:], src_ap,
                            ogidx_sb[:, c0 * P // 16:(c0 + nch) * P // 16],
                            nidx, nidx, P,
                            single_packet=False, queue_num=0)
                        ov_ready[g].append((b, c0, nch, msg))

                def emit_ov_compute(g):
                    for (b, c0, nch, msg) in ov_ready.pop(g):
                        osel = ovselp.tile([P, nch, H], dt.float16,
                                           tag=f"osel{b}", name=f"osel_g{g}b{b}")
                        nc.vector.tensor_copy(osel[:], msg[:, :, H:2 * H])
                        nc.vector.copy_predicated(
                            osel[:],
                            oparity_sb[:, c0:c0 + nch].unsqueeze(2)
                            .to_broadcast([P, nch, H]),
                            msg[:, :, 0:H])
                        oh = ovohp.tile([P, nch, P], dt.float16, tag=f"ooh{b}",
                                        name=f"ooh_g{g}b{b}")
                        nc.vector.tensor_tensor(
                            oh[:],
                            odstloc_sb[:, c0:c0 + nch].unsqueeze(2)
                            .to_broadcast([P, nch, P]),
                            iota[:, :].unsqueeze(1).to_broadcast([P, nch, P]),
                            mybir.AluOpType.is_equal,
                        )
                        for ci in range(c0, c0 + nch):
                            _, bb, t = ov_sched["chunk_meta"][ci]
                            if t not in ov_acc:
                                ov_acc[t] = ovaccp.tile(
                                    [P, H], dt.float32, tag=f"oacc{t % OVGROUP}",
                                    name=f"oacc_t{t}")
                            nc.tensor.matmul(
                                ov_acc[t][:],
                                lhsT=oh[:, ci - c0, :],
                                rhs=osel[:, ci - c0, :],
                                start=(ci == ov_first[t]),
                                stop=False,
                            )

                emit_ov_gathers(0)
                for g in range(ngroups):
                    t0 = g * GROUP
                    t1 = t0 + gsz[g]
                    ka, kb = KA[g], KB[g]
                    nt = gsz[g]
                    # slab gathers (plane A then B, split into half-groups)
                    msgs = []
                    for (pp, kk, base, idx_sb) in (
                        (0, ka, baseA[g], eidxa_sb),
                        (1, kb, baseB[g], eidxb_sb),
                    ):
                        msg = smsgp.tile([P, nt, kk, P], dt.float16,
                                         tag=f"smsg{pp}", name=f"smsg_g{g}p{pp}")
                        src_ap = g_pairs if pp == 0 else g_pairs[HR:, :]
                        nch = nt * kk
                        nc.gpsimd.dma_gather(
                            msg[:].rearrange("p t k e -> p (t k) e"),
                            src_ap,
                            idx_sb[:, base * P // 16:(base + nch) * P // 16],
                            nch * P, nch * P, P,
                            single_packet=False, queue_num=0)
                        msgs.append(msg)
                    if g + 1 < ngroups:
                        emit_ov_gathers(g + 1)

                    ksum = ka + kb
                    sel = sselp.tile([P, nt, ksum, H], dt.float16, tag="sel",
                                     name=f"sel_g{g}")
                    for (pp, kk, base, msk_sb, kofs) in (
                        (0, ka, baseA[g], emska_sb, 0),
                        (1, kb, baseB[g], emskb_sb, ka),
                    ):
                        seg = sel[:, :, kofs:kofs + kk, :]
                        nc.vector.tensor_copy(seg, msgs[pp][:, :, :, H:2 * H])
                        mview = (
                            msk_sb[:, base:base + nt * kk]
                            .rearrange("p (t k) -> p t k", t=nt)
                            .unsqueeze(3).to_broadcast([P, nt, kk, H])
                        )
                        nc.vector.copy_predicated(
                            seg, mview, msgs[pp][:, :, :, 0:H])

                    emit_ov_compute(g)

                    m = ksum
                    while m > 1:
                        hh = m // 2
                        nc.vector.tensor_tensor(
                            sel[:, :, 0:hh, :], sel[:, :, 0:hh, :],
                            sel[:, :, m - hh:m, :], mybir.AluOpType.add)
                        m = m - hh

                    emb = sembp.tile([P, nt, H], dt.float16, name=f"emb_g{g}")
                    for tr in range(nt):
                        a = ov_acc.pop(t0 + tr)
                        nc.tensor.matmul(
                            a[:], lhsT=ident[:], rhs=sel[:, tr, 0, :],
                            start=False, stop=False)
                        nc.tensor.matmul(
                            a[:], lhsT=ident[:],
                            rhs=bcd_sb[:, t0 + tr, :],
                            start=False, stop=True)
                        nc.scalar.activation(
                            emb[:, tr, :], a[:],
                            mybir.ActivationFunctionType.Lrelu,
                            scale=dinv[:, t0 + tr:t0 + tr + 1],
                            alpha=NEG)
                    nc.sync.dma_start(
                        e_shard[t0 * P:t1 * P, :].rearrange(
                            "(t p) f -> p t f", p=P),
                        emb[:])

            nc.gpsimd.collective_compute(
                "AllGather", mybir.AluOpType.bypass,
                replica_groups=[list(range(NC))],
                ins=[e_shard[:, :].opt()],
                outs=[e_full[:, :].opt()],
            )

            if debug:
                nc.sync.dma_start(dbg_e[:, :], e_full[:, :])

            # ---------------- phase D: pair MLP ----------------
            with (
                tc.tile_pool(name="pconst", bufs=1) as pcpool,
                tc.tile_pool(name="pwidx", bufs=2) as pwidxp,
                tc.tile_pool(name="pmsg", bufs=2) as pmsgp,
                tc.tile_pool(name="poh", bufs=2) as pohp,
                tc.tile_pool(name="pacc", bufs=1, space="PSUM") as paccp,
                tc.tile_pool(name="pxs", bufs=1) as pxsp,
                tc.tile_pool(name="ptps", bufs=2, space="PSUM") as ptps,
                tc.tile_pool(name="pzps", bufs=1, space="PSUM") as pzps,
                tc.tile_pool(name="pops", bufs=1, space="PSUM") as pops,
                tc.tile_pool(name="psb", bufs=4) as psbp,
            ):
                pdstloc_sb = pcpool.tile([P, psched["totchunks"]], dt.float16)
                nc.sync.dma_start(pdstloc_sb[:], pdstloc_in[:, :])
                xs_sb = pxsp.tile([P, psched["ntiles"], H], dt.float16)

                def consume_pair(st, a):
                    nc.scalar.activation(
                        xs_sb[:, st, :], a[:],
                        mybir.ActivationFunctionType.Copy)

                _emit_scatter(nc, dt, e_pairs, pgidx_in, pdstloc_sb, iota,
                              psched, (pwidxp, pmsgp, pohp, paccp),
                              consume_pair, "p")

                for k in range(PCH):
                    xt_ps = ptps.tile([P, P], dt.float16)
                    nc.tensor.transpose(xt_ps[0:H, :], xs_sb[:, k, :], ident[:])
                    nc.tensor.transpose(xt_ps[H:P, :], xs_sb[:, PCH + k, :], ident[:])
                    xijt = psbp.tile([P, P], dt.float16, tag="xijt")
                    nc.scalar.activation(
                        xijt[:], xt_ps[:], mybir.ActivationFunctionType.Copy)
                    z_ps = pzps.tile([16, P], dt.float32)
                    nc.tensor.matmul(z_ps[:], lhsT=w1_sb[:], rhs=xijt[:],
                                     start=True, stop=True)
                    z2 = psbp.tile([16, P], dt.float32, tag="z2")
                    nc.scalar.activation(
                        z2[:], z_ps[:], mybir.ActivationFunctionType.Lrelu,
                        bias=b1_sb[:, 0:1], alpha=NEG)
                    o_ps = pops.tile([1, P], dt.float32)
                    nc.tensor.matmul(o_ps[:], lhsT=w2_sb[:], rhs=z2[:],
                                     start=True, stop=True)
                    osb = psbp.tile([1, P], dt.float32, tag="osb")
                    nc.scalar.activation(
                        osb[:], o_ps[:], mybir.ActivationFunctionType.Sigmoid,
                        bias=b2_sb[:, 0:1], scale=1.0)
                    nc.sync.dma_start(
                        outp[k * P:(k + 1) * P, :].rearrange("r one -> one r"),
                        osb[0:1, :])

    # align each gather's SWDGE queue with its Tile-assigned DMA lane so
    # semaphore<->queue locking stays consistent (4-way parallel desc gen)
    for blk in nc.m.functions[0].blocks:
        for inst in blk.instructions:
            if isinstance(inst, mybir.InstDMAGatherAnt):
                si = inst.sync_info
                for u in (si.on_update if si else []):
                    mm = re.match(r"DMASW(\d+)_", u.ant_name or "")
                    if mm:
                        inst.queue_num = int(mm.group(1)) % 4
                        break

    nc.compile()
    return nc


def kernel(**inputs) -> np.ndarray:
    in_maps, sched = _prep(inputs)
    nc = _build(sched)
    res = run_bass_kernel_spmd(nc, in_maps, list(range(NC)))
    out = np.concatenate([res.results[c]["out"] for c in range(NC)], axis=0)
    return out.astype(np.float32)


# revision 22
# speedup vs baseline: 1.3305x; 1.3305x over previous
"""GCN message-passing kernel for 8 Trainium2 NeuronCores (Bass/Tile).

Computes (matching the jax reference):
    h = x @ W_conv                      [N, H]
    node_embed = leaky_relu(D^-1/2 (A+I) D^-1/2 h + b_conv)
    out = sigmoid(leaky(cat(e[i], e[j]) @ W1 + b1) @ W2 + b2)

Distribution: nodes are sharded over the 8 cores (dst-sharded aggregation
with an AllGather of the scaled features g = dinv * h). Edges are
partitioned by destination shard and scatter-added on the TensorEngine
via per-destination-tile one-hot matmuls; per-edge source rows are
fetched with bulk dma_gather (pair-packed fp16 rows, 4 SWDGE queues).
The pair-MLP head reuses the same gather + one-hot-permute machinery to
collect node embeddings in pair order.
"""

import re

import numpy as np

import concourse.bass as bass
import concourse.bacc as bacc
import concourse.mybir as mybir
import concourse.tile as tile
from concourse import library_config
from concourse.bass_utils import run_bass_kernel_spmd

NC = 8
N_NODES = 100000
F_IN = 256
H = 64
NEG = 0.01

P = 128                    # partitions / tile height
TILES = 98                 # node tiles per core
SHARD = TILES * P          # 12544 nodes per core
NPAD = NC * SHARD          # 100352
GROUP = 4                  # node tiles per edge gather group
NBUCKET = 4                # (src range) x (src parity)
PGROUP = 4                 # pair slot-tiles per gather group


def half_range():
    return NPAD // 4       # 25088: int16-addressable pair rows per range


def _wrap_idx_window(idx):
    """int array [W] (W % 16 == 0) -> [128, W//16] int16 wrapped/replicated."""
    w = idx.reshape(-1, 16).T.astype(np.int16)
    return np.tile(w, (8, 1))


def _scatter_sched(core, tl, loc, node, ntiles, group_sz):
    """Build a cross-core-uniform gather/scatter schedule.

    core/tl/loc/node: int arrays over items. Each item is gathered from
    pair-packed row (node>>1) and scatter-added to column `loc` of tile
    `tl` on core `core`.
    Returns (sched_dict, idx_i16 [NC,128,totidx//16], loc_f16 [NC,128,totchunks]).
    """
    HR = half_range()
    ngroups = (ntiles + group_sz - 1) // group_sz
    bucket = 2 * (node >= 2 * HR).astype(np.int64) + (node & 1)
    grp = tl // group_sz

    tid = ((core * ngroups + grp) * NBUCKET + bucket) * ntiles + tl
    n_bins = NC * ngroups * NBUCKET * ntiles
    cnt = np.bincount(tid, minlength=n_bins).reshape(NC, ngroups, NBUCKET, ntiles)
    K = (cnt + P - 1) // P
    K = K.max(axis=0)                       # [ngroups, NBUCKET, ntiles]
    per_tile = K.sum(axis=1)                # [ngroups, ntiles]
    for t in range(ntiles):
        g = t // group_sz
        if per_tile[g, t] == 0:
            K[g, 0, t] = 1

    chunk_meta = []
    win_meta = []
    for g in range(ngroups):
        t0, t1 = g * group_sz, min((g + 1) * group_sz, ntiles)
        for b in range(NBUCKET):
            c0 = len(chunk_meta)
            for t in range(t0, t1):
                for _ in range(K[g, b, t]):
                    chunk_meta.append((g, b, t))
            win_meta.append((g, b, c0, len(chunk_meta) - c0))
    totchunks = len(chunk_meta)
    totidx = totchunks * P

    slot_base = np.zeros((ngroups, NBUCKET, ntiles), np.int64)
    acc = 0
    for g in range(ngroups):
        t0, t1 = g * group_sz, min((g + 1) * group_sz, ntiles)
        for b in range(NBUCKET):
            for t in range(t0, t1):
                slot_base[g, b, t] = acc
                acc += K[g, b, t] * P
    assert acc == totidx

    loc_arr = np.full((NC, totidx), 255, np.int64)
    pidx_arr = np.zeros((NC, totidx), np.int64)
    order = np.lexsort((tl, bucket, grp, core))
    so_core = core[order]
    so_tid = tid[order]
    so_node = node[order]
    so_loc = loc[order]
    so_bucket = bucket[order]
    so_grp = grp[order]
    so_t = tl[order]
    starts = np.r_[0, np.flatnonzero(np.diff(so_tid)) + 1]
    run_ids = np.zeros(len(so_tid), np.int64)
    run_ids[starts[1:]] = 1
    run_ids = np.cumsum(run_ids)
    rank = np.arange(len(so_tid)) - starts[run_ids]
    slot = slot_base[so_grp, so_bucket, so_t] + rank
    pair_local = (so_node >> 1) - (so_bucket >= 2) * HR
    loc_arr[so_core, slot] = so_loc
    pidx_arr[so_core, slot] = pair_local

    loc_f16 = np.zeros((NC, P, totchunks), np.float16)
    idx_i16 = np.zeros((NC, P, totidx // 16), np.int16)
    for c in range(NC):
        loc_f16[c] = loc_arr[c].reshape(totchunks, P).T.astype(np.float16)
        for (g, b, c0, nch) in win_meta:
            if nch == 0:
                continue
            lo, hi = c0 * P, (c0 + nch) * P
            idx_i16[c][:, lo // 16: hi // 16] = _wrap_idx_window(pidx_arr[c, lo:hi])

    sched = {
        "chunk_meta": chunk_meta,
        "win_meta": win_meta,
        "totchunks": totchunks,
        "totidx": totidx,
        "ntiles": ntiles,
        "ngroups": ngroups,
        "group_sz": group_sz,
    }
    return sched, idx_i16, loc_f16


def _prep(inputs):
    x = np.asarray(inputs["x"], np.float32)
    edge_index = np.asarray(inputs["edge_index"], np.int64)
    index = np.asarray(inputs["index"], np.int64)
    W_conv = np.asarray(inputs["W_conv"], np.float32)
    b_conv = np.asarray(inputs["b_conv"], np.float32)
    W1 = np.asarray(inputs["W1"], np.float32)
    b1 = np.asarray(inputs["b1"], np.float32)
    W2 = np.asarray(inputs["W2"], np.float32)
    b2 = np.asarray(inputs["b2"], np.float32)

    n = x.shape[0]
    src = edge_index[0].astype(np.int64)
    dst = edge_index[1].astype(np.int64)
    loops = np.arange(n, dtype=np.int64)
    src = np.concatenate([src, loops])
    dst = np.concatenate([dst, loops])

    deg = np.bincount(dst, minlength=NPAD).astype(np.float32)
    deg[n:] = 1.0

    esched, eidx, eloc = _scatter_sched(
        core=dst // SHARD, tl=(dst % SHARD) // P, loc=dst % P, node=src,
        ntiles=TILES, group_sz=GROUP)

    # pair stream: per core PB pairs; side slots [xi: 0..PB) [xj: PB..2PB)
    B = index.shape[0]
    PB = B // NC
    assert PB % P == 0
    PCH = PB // P
    pair_global = np.arange(B, dtype=np.int64)
    pcore = pair_global // PB
    plocal = pair_global % PB
    s_core = np.concatenate([pcore, pcore])
    s_slot = np.concatenate([plocal, PB + plocal])
    s_node = np.concatenate([index[:, 0], index[:, 1]]).astype(np.int64)
    psched, pidx, ploc = _scatter_sched(
        core=s_core, tl=s_slot // P, loc=s_slot % P, node=s_node,
        ntiles=2 * PCH, group_sz=PGROUP)

    xpad = np.zeros((NPAD, F_IN), np.float32)
    xpad[:n] = x
    xT = xpad.T.astype(np.float16)
    xT_shards = [
        np.ascontiguousarray(
            xT[:, c * SHARD:(c + 1) * SHARD].reshape(2, P, SHARD).transpose(1, 0, 2)
        ) for c in range(NC)
    ]
    deg_sb = [
        np.ascontiguousarray(
            deg[c * SHARD:(c + 1) * SHARD].reshape(TILES, P).T
        ) for c in range(NC)
    ]
    bcd = (np.sqrt(deg)[:, None] * b_conv[None, :]).astype(np.float16)
    bcd_sh = [
        np.ascontiguousarray(
            bcd[c * SHARD:(c + 1) * SHARD].reshape(TILES, P, H)
            .transpose(1, 0, 2)
        ) for c in range(NC)
    ]

    consts = {
        "wc": np.ascontiguousarray(
            W_conv.reshape(2, P, H).transpose(1, 0, 2)).astype(np.float16),
        "bconvb": np.broadcast_to(b_conv, (P, H)).astype(np.float32).copy(),
        "iota": np.broadcast_to(np.arange(P, dtype=np.float16), (P, P)).copy(),
        "ident": np.eye(P, dtype=np.float16),
        "w1": W1.astype(np.float16),
        "b1": b1.reshape(16, 1).astype(np.float32),
        "w2": W2.astype(np.float32),
        "b2t": b2.reshape(1, 1).astype(np.float32),
    }
    sched = {"edge": esched, "pair": psched, "PCH": PCH}
    in_maps = []
    for c in range(NC):
        m = {
            "xt": xT_shards[c],
            "deg": deg_sb[c],
            "bcd": bcd_sh[c],
            "edstloc": eloc[c],
            "egidx": eidx[c],
            "pdstloc": ploc[c],
            "pgidx": pidx[c],
        }
        m.update(consts)
        in_maps.append(m)
    return in_maps, sched


def _emit_scatter(nc, dt, src_pairs, idx_sb, loc_sb, iota, sched,
                  pools, consume, prefix):
    """Gather pair-packed rows per window, build one-hot, matmul-accumulate
    per destination tile in PSUM, then hand each finished tile to consume.
    idx_sb is an SBUF-resident int16 index table."""
    msgp, ohp, accp = pools
    chunk_meta = sched["chunk_meta"]
    win_meta = sched["win_meta"]
    ngroups = sched["ngroups"]
    ntiles = sched["ntiles"]
    group_sz = sched["group_sz"]
    HR = half_range()

    first_chunk = {}
    last_chunk = {}
    for ci, (g, b, t) in enumerate(chunk_meta):
        first_chunk.setdefault(t, ci)
        last_chunk[t] = ci

    acc_tiles = {}
    for g in range(ngroups):
        t0, t1 = g * group_sz, min((g + 1) * group_sz, ntiles)
        for (gg, b, c0, nch) in win_meta:
            if gg != g or nch == 0:
                continue
            nidx = nch * P
            msg = msgp.tile([P, nch, P], dt.float16, tag=f"{prefix}msg{b}",
                            name=f"{prefix}msg_g{g}b{b}")
            src_ap = src_pairs if b < 2 else src_pairs[HR:, :]
            nc.gpsimd.dma_gather(
                msg[:], src_ap,
                idx_sb[:, c0 * P // 16:(c0 + nch) * P // 16],
                nidx, nidx, P,
                single_packet=False, queue_num=0)
            oh = ohp.tile([P, nch, P], dt.float16, tag=f"{prefix}oh{b}",
                          name=f"{prefix}oh_g{g}b{b}")
            nc.vector.tensor_tensor(
                oh[:],
                loc_sb[:, c0:c0 + nch].unsqueeze(2).to_broadcast([P, nch, P]),
                iota[:, :].unsqueeze(1).to_broadcast([P, nch, P]),
                mybir.AluOpType.is_equal,
            )
            for ci in range(c0, c0 + nch):
                _, bb, t = chunk_meta[ci]
                if t not in acc_tiles:
                    acc_tiles[t] = accp.tile(
                        [P, H], dt.float32, tag=f"{prefix}acc{t % group_sz}",
                        name=f"{prefix}acc_t{t}")
                par = bb & 1
                nc.tensor.matmul(
                    acc_tiles[t][:],
                    lhsT=oh[:, ci - c0, :],
                    rhs=msg[:, ci - c0, par * H:(par + 1) * H],
                    start=(ci == first_chunk[t]),
                    stop=False,
                )
        for t in range(t0, t1):
            consume(t, acc_tiles.pop(t))


def _build(sched, debug=False, passes=1):
    dt = mybir.dt
    esched = sched["edge"]
    psched = sched["pair"]
    PCH = sched["PCH"]

    nc = bacc.Bacc("TRN2", target_bir_lowering=False, debug=False,
                   enable_asserts=False, num_devices=NC, num_swdge_queues=4)

    xt_in = nc.dram_tensor("xt", [P, 2, SHARD], dt.float16, kind="ExternalInput")
    deg_in = nc.dram_tensor("deg", [P, TILES], dt.float32, kind="ExternalInput")
    bcd_in = nc.dram_tensor("bcd", [P, TILES, H], dt.float16, kind="ExternalInput")
    edstloc_in = nc.dram_tensor("edstloc", [P, esched["totchunks"]], dt.float16,
                                kind="ExternalInput")
    egidx_in = nc.dram_tensor("egidx", [P, esched["totidx"] // 16], dt.int16,
                              kind="ExternalInput")
    pdstloc_in = nc.dram_tensor("pdstloc", [P, psched["totchunks"]], dt.float16,
                                kind="ExternalInput")
    pgidx_in = nc.dram_tensor("pgidx", [P, psched["totidx"] // 16], dt.int16,
                              kind="ExternalInput")
    wc_in = nc.dram_tensor("wc", [P, 2, H], dt.float16, kind="ExternalInput")
    bconvb_in = nc.dram_tensor("bconvb", [P, H], dt.float32, kind="ExternalInput")
    iota_in = nc.dram_tensor("iota", [P, P], dt.float16, kind="ExternalInput")
    ident_in = nc.dram_tensor("ident", [P, P], dt.float16, kind="ExternalInput")
    w1_in = nc.dram_tensor("w1", [P, 16], dt.float16, kind="ExternalInput")
    b1_in = nc.dram_tensor("b1", [16, 1], dt.float32, kind="ExternalInput")
    w2_in = nc.dram_tensor("w2", [16, 1], dt.float32, kind="ExternalInput")
    b2_in = nc.dram_tensor("b2t", [1, 1], dt.float32, kind="ExternalInput")
    outp = nc.dram_tensor("out", [PCH * P, 1], dt.float32, kind="ExternalOutput")
    if debug:
        dbg_g = nc.dram_tensor("dbg_g", [NPAD, H], dt.float16, kind="ExternalOutput")
        dbg_e = nc.dram_tensor("dbg_e", [NPAD, H], dt.float16, kind="ExternalOutput")

    g_shard = nc.dram_tensor("g_shard", [SHARD, H], dt.float16)
    g_full = nc.dram_tensor("g_full", [NPAD, H], dt.float16, addr_space="Shared")
    e_shard = nc.dram_tensor("e_shard", [SHARD, H], dt.float16)
    e_full = nc.dram_tensor("e_full", [NPAD, H], dt.float16)

    g_pairs = g_full[:, :].rearrange("(r two) f -> r (two f)", two=2)
    e_pairs = e_full[:, :].rearrange("(r two) f -> r (two f)", two=2)

    with tile.TileContext(nc) as tc:
        nc.gpsimd.load_library(library_config.mlp)

        with (
            tc.tile_pool(name="const", bufs=1) as cpool,
            tc.tile_pool(name="dinvp", bufs=1) as dpool,
        ):
            wc_sb = cpool.tile([P, 2, H], dt.float16)
            nc.sync.dma_start(wc_sb[:], wc_in[:, :, :])
            bconvb = cpool.tile([P, H], dt.float32)
            nc.sync.dma_start(bconvb[:], bconvb_in[:, :])
            iota = cpool.tile([P, P], dt.float16)
            nc.sync.dma_start(iota[:], iota_in[:, :])
            ident = cpool.tile([P, P], dt.float16)
            nc.sync.dma_start(ident[:], ident_in[:, :])
            w1_sb = cpool.tile([P, 16], dt.float16)
            nc.sync.dma_start(w1_sb[:], w1_in[:, :])
            b1_sb = cpool.tile([16, 1], dt.float32)
            nc.sync.dma_start(b1_sb[:], b1_in[:, :])
            w2_sb = cpool.tile([16, 1], dt.float32)
            nc.sync.dma_start(w2_sb[:], w2_in[:, :])
            b2_sb = cpool.tile([1, 1], dt.float32)
            nc.sync.dma_start(b2_sb[:], b2_in[:, :])
            edstloc_sb = cpool.tile([P, esched["totchunks"]], dt.float16)
            nc.sync.dma_start(edstloc_sb[:], edstloc_in[:, :])
            egidx_sb = cpool.tile([P, esched["totidx"] // 16], dt.int16)
            nc.sync.dma_start(egidx_sb[:], egidx_in[:, :])
            pgidx_sb = cpool.tile([P, psched["totidx"] // 16], dt.int16)
            nc.sync.dma_start(pgidx_sb[:], pgidx_in[:, :])
            bcd_sb = cpool.tile([P, TILES, H], dt.float16)
            nc.sync.dma_start(bcd_sb[:], bcd_in[:, :, :])
            zero64_sb = cpool.tile([P, H], dt.float16)
            nc.vector.memset(zero64_sb[:], 0.0)

            deg_sb = dpool.tile([P, TILES], dt.float32)
            nc.sync.dma_start(deg_sb[:], deg_in[:, :])
            sq = dpool.tile([P, TILES], dt.float32)
            nc.scalar.activation(sq[:], deg_sb[:], mybir.ActivationFunctionType.Sqrt)
            dinv = dpool.tile([P, TILES], dt.float32)
            nc.vector.reciprocal(dinv[:], sq[:])

            def _one_pass():
                # ---------------- phase A: g = (x @ W) * dinv ----------------
                XBLK = 16
                with (
                    tc.tile_pool(name="xtp", bufs=2) as xtp,
                    tc.tile_pool(name="hps", bufs=4, space="PSUM") as hps,
                    tc.tile_pool(name="gsb", bufs=1) as gsbp,
                ):
                    g_sb = gsbp.tile([P, TILES, H], dt.float16)
                    for blk in range((TILES + XBLK - 1) // XBLK):
                        t0, t1 = blk * XBLK, min((blk + 1) * XBLK, TILES)
                        xt_sb = xtp.tile([P, 2, (t1 - t0) * P], dt.float16, tag="xt")
                        nc.sync.dma_start(xt_sb[:], xt_in[:, :, t0 * P: t1 * P])
                        for t in range(t0, t1):
                            h_ps = hps.tile([P, H], dt.float32)
                            for k in range(2):
                                nc.tensor.matmul(
                                    h_ps[:],
                                    lhsT=xt_sb[:, k, (t - t0) * P:(t - t0 + 1) * P],
                                    rhs=wc_sb[:, k, :],
                                    start=(k == 0), stop=(k == 1),
                                )
                            nc.scalar.activation(
                                g_sb[:, t, :], h_ps[:],
                                mybir.ActivationFunctionType.Copy,
                                scale=dinv[:, t:t + 1],
                            )
                    nc.sync.dma_start(
                        g_shard[:, :].rearrange("(t p) f -> p t f", p=P),
                        g_sb[:, :, :],
                    )

                nc.gpsimd.collective_compute(
                    "AllGather", mybir.AluOpType.bypass,
                    replica_groups=[list(range(NC))],
                    ins=[g_shard[:, :].opt()],
                    outs=[g_full[:, :].opt()],
                )

                # ---------------- phase C: aggregate per dst tile ----------------
                with (
                    tc.tile_pool(name="emsg", bufs=2) as msgp,
                    tc.tile_pool(name="eoh", bufs=2) as ohp,
                    tc.tile_pool(name="eacc", bufs=2, space="PSUM") as accp,
                    tc.tile_pool(name="epost", bufs=4) as postp,
                    tc.tile_pool(name="eemb", bufs=4) as embp,
                ):
                    def consume_edge(t, a):
                        nc.tensor.matmul(
                            a[:], lhsT=ident[:], rhs=bcd_sb[:, t, :],
                            start=False, stop=True)
                        emb = embp.tile([P, H], dt.float16, name=f"emb_{t}")
                        nc.scalar.activation(
                            emb[:], a[:], mybir.ActivationFunctionType.Lrelu,
                            scale=dinv[:, t:t + 1], alpha=NEG)
                        nc.sync.dma_start(e_shard[t * P:(t + 1) * P, :], emb[:])

                    _emit_scatter(nc, dt, g_pairs, egidx_sb, edstloc_sb, iota,
                                  esched, (msgp, ohp, accp), consume_edge, "e")

                nc.gpsimd.collective_compute(
                    "AllGather", mybir.AluOpType.bypass,
                    replica_groups=[list(range(NC))],
                    ins=[e_shard[:, :].opt()],
                    outs=[e_full[:, :].opt()],
                )

                if debug:
                    nc.sync.dma_start(dbg_g[:, :], g_full[:, :])
                    nc.sync.dma_start(dbg_e[:, :], e_full[:, :])

                # ---------------- phase D: pair MLP ----------------
                with (
                    tc.tile_pool(name="pconst", bufs=1) as pcpool,
                        tc.tile_pool(name="pmsg", bufs=2) as pmsgp,
                    tc.tile_pool(name="poh", bufs=2) as pohp,
                    tc.tile_pool(name="pacc", bufs=1, space="PSUM") as paccp,
                    tc.tile_pool(name="pxs", bufs=1) as pxsp,
                    tc.tile_pool(name="ptps", bufs=2, space="PSUM") as ptps,
                    tc.tile_pool(name="pzps", bufs=1, space="PSUM") as pzps,
                    tc.tile_pool(name="pops", bufs=1, space="PSUM") as pops,
                    tc.tile_pool(name="psb", bufs=4) as psbp,
                ):
                    pdstloc_sb = pcpool.tile([P, psched["totchunks"]], dt.float16)
                    nc.sync.dma_start(pdstloc_sb[:], pdstloc_in[:, :])
                    xs_sb = pxsp.tile([P, psched["ntiles"], H], dt.float16)

                    def consume_pair(st, a):
                        nc.vector.tensor_copy(xs_sb[:, st, :], a[:])

                    _emit_scatter(nc, dt, e_pairs, pgidx_sb, pdstloc_sb, iota,
                                  psched, (pmsgp, pohp, paccp),
                                  consume_pair, "p")

                    for k in range(PCH):
                        xt_ps = ptps.tile([P, P], dt.float16)
                        nc.tensor.transpose(xt_ps[0:H, :], xs_sb[:, k, :], ident[:])
                        nc.tensor.transpose(xt_ps[H:P, :], xs_sb[:, PCH + k, :], ident[:])
                        xijt = psbp.tile([P, P], dt.float16, tag="xijt")
                        nc.vector.tensor_copy(xijt[:], xt_ps[:])
                        z_ps = pzps.tile([16, P], dt.float32)
                        nc.tensor.matmul(z_ps[:], lhsT=w1_sb[:], rhs=xijt[:],
                                         start=True, stop=True)
                        zb = psbp.tile([16, P], dt.float32, tag="zb")
                        nc.vector.tensor_scalar(
                            zb[:], z_ps[:], b1_sb[:, 0:1], None, mybir.AluOpType.add)
                        m2 = psbp.tile([16, P], dt.float32, tag="m2")
                        nc.scalar.activation(
                            m2[:], zb[:], mybir.ActivationFunctionType.Copy,
                            bias=0.0, scale=NEG)
                        z2 = psbp.tile([16, P], dt.float32, tag="z2")
                        nc.vector.tensor_tensor(z2[:], zb[:], m2[:], mybir.AluOpType.max)
                        o_ps = pops.tile([1, P], dt.float32)
                        nc.tensor.matmul(o_ps[:], lhsT=w2_sb[:], rhs=z2[:],
                                         start=True, stop=True)
                        osb = psbp.tile([1, P], dt.float32, tag="osb")
                        nc.scalar.activation(
                            osb[:], o_ps[:], mybir.ActivationFunctionType.Sigmoid,
                            bias=b2_sb[:, 0:1], scale=1.0)
                        nc.sync.dma_start(
                            outp[k * P:(k + 1) * P, :].rearrange("r one -> one r"),
                            osb[0:1, :])

            for _ in range(passes):
                _one_pass()

    # align each gather's SWDGE queue with its Tile-assigned DMA lane so
    # semaphore<->queue locking stays consistent (4-way parallel desc gen)
    for blk in nc.m.functions[0].blocks:
        for inst in blk.instructions:
            if isinstance(inst, mybir.InstDMAGatherAnt):
                si = inst.sync_info
                for u in (si.on_update if si else []):
                    mm = re.match(r"DMASW(\d+)_", u.ant_name or "")
                    if mm:
                        inst.queue_num = int(mm.group(1)) % 4
                        break

    nc.compile()
    return nc


def kernel(**inputs) -> np.ndarray:
    in_maps, sched = _prep(inputs)
    nc = _build(sched)
    res = run_bass_kernel_spmd(nc, in_maps, list(range(NC)))
    out = np.concatenate([res.results[c]["out"] for c in range(NC)], axis=0)
    return out.astype(np.float32)

